# revision 3
# baseline (speedup 1.0000x reference)
"""Trainium2 Bass kernel: DGCNN Zernike-monomial interwiner (nn_DGCNN_8839042695322).

Computes, per point p=(x,y,z):
  out[.., 16, 4] = concat_l( einsum(zernike_monoms(p)[l], Wl) ) for l=0..3
Every output channel is a degree<=3 polynomial in (x,y,z); weights are folded
host-side into per-channel scalars.

Memory-bound problem: the f32 output is 32 MB/core. The device computes and
stores everything in fp16 (norm rel err ~6e-4, far under the 2e-2 gate),
halving HBM write traffic and enabling the DVE 2x perf mode. The SBUF output
tile is CHANNEL-major [P, 64, T] so all compute writes are unit-stride; the
DRAM layout is tile-blocked and the host does the final (T, 64) transpose
during unsharding. x is pre-transposed on host to [P, 3, COLS] fp16.

All shared polynomial planes live in one row-mapped tile PL so related
planes can be produced by single multi-row DVE ops (merged squares,
products, pair-muls). The 48 "simple" output channels (base_m * w[m,u])
are split per-tile across DVE (tensor_scalar, 2x mode), ACT (Copy+scale)
and GPSIMD (tensor_scalar) to use all three elementwise engines.

Sharding: pure data parallel over the batch axis across 8 NeuronCores.
"""

import numpy as np

import concourse.bacc as bacc
import concourse.tile as tile
from concourse import mybir
from concourse.bass_utils import run_bass_kernel_spmd

# Problem geometry (hardcoded per spec: x [32, 32768, 3] f32, 8 cores).
B, N, M_CORES = 32, 32768, 8
PTS_PER_CORE = B * N // M_CORES  # 131072
P = 128                          # SBUF partitions
COLS = PTS_PER_CORE // P         # 1024 points per partition

# Schedule: (T, n_act_groups, n_gp_groups, use_mega) per tile.
# ACT/GP counts are in 4-channel groups of the 12 "simple" groups m=4..15;
# the rest go to DVE (mega broadcast op if use_mega else per-channel TS).
SCHEDULE = [
    (64, 4, 0, True),
    (192, 5, 1, False),
    (320, 6, 2, False),
    (448, 6, 2, False),
]
assert sum(s[0] for s in SCHEDULE) == COLS
# ACT takes groups from the front of this list (bases available earliest),
# GPSIMD from the back, DVE keeps the middle.
ACT_PRIORITY = [4, 5, 7, 6, 10, 14, 8, 15, 9, 11, 12, 13]

# Real spherical-harmonic constants (match reference).
C0 = 0.28209479177387814
C1 = 0.4886025119029199
C2_XY = 1.0925484305920792
C2_0 = 0.31539156525252005
C2_2 = 0.5462742152960396
C3_3 = 0.5900435899266435
C3_2 = 2.890611442640554
C3_1 = 0.4570457994644658
C3_0 = 0.3731763325901154
C3_P2 = 1.445305721320277

# PL plane-tile row map. Rows 9..20 hold the bases for channel groups
# m=4..15 in m-order so a contiguous m-range is one strided AP.
R_X2, R_Y2, R_Z2 = 0, 1, 2
R_N2A, R_N2 = 3, 4
R_A3, R_C3, R_B3, R_D3 = 5, 6, 7, 8
BASE0 = 9  # + (m-4): xy yz t2a zx x2my2 ya xyz yc zd xc zxmy xb
R_SP = 21  # ..24
R_PL = 25

_cache: dict = {}


def _host_constants(W0, b0, W1, W2, W3):
    """Fold interwiner weights into per-channel scalars (see baseline)."""
    coef2 = np.array([C2_XY, C2_XY, C2_0, C2_XY, C2_2], dtype=np.float64)
    # base for m12 is d3 = z2 - 0.6*n2 = (2z2-3x2-3y2)/5, so fold the 5 in.
    coef3 = np.array(
        [C3_3, C3_2, C3_1, 5.0 * C3_0, C3_1, C3_P2, C3_3], dtype=np.float64
    )
    w2 = (coef2[:, None] * W2[0][None, :].astype(np.float64)).astype(np.float32)
    w3 = (coef3[:, None] * W3[0][None, :].astype(np.float64)).astype(np.float32)
    A0 = (C0 * W0[0].astype(np.float64) + b0.astype(np.float64)).astype(np.float32)
    B0 = (C0 * W0[1].astype(np.float64)).astype(np.float32)
    AA1 = (C1 * W1[0].astype(np.float64)).astype(np.float32)
    BB1 = (C1 * W1[1].astype(np.float64)).astype(np.float32)
    w64 = np.zeros(64, np.float32)
    w64[16:36] = w2.reshape(-1)
    w64[36:64] = w3.reshape(-1)
    return dict(A0=A0, B0=B0, AA1=AA1, BB1=BB1, w2=w2, w3=w3, w64=w64)


def _build_program(consts, schedule):
    dt = mybir.dt.float16
    F = mybir.ActivationFunctionType
    ALU = mybir.AluOpType
    A0, B0 = consts["A0"], consts["B0"]
    AA1, BB1 = consts["AA1"], consts["BB1"]
    w2, w3 = consts["w2"], consts["w3"]
    wrow = {m: w2[m - 4] for m in range(4, 9)}
    wrow.update({m: w3[m - 9] for m in range(9, 16)})

    need_mega = any(s[3] for s in schedule)

    nc = bacc.Bacc(
        "TRN2", target_bir_lowering=False, debug=False, num_devices=M_CORES
    )
    xin = nc.dram_tensor("xin", [P, 3, COLS], dt, kind="ExternalInput").ap()
    yout = nc.dram_tensor("yout", [P, COLS * 64], dt, kind="ExternalOutput").ap()
    if need_mega:
        wvec = nc.dram_tensor("wvec", [1, 64], dt, kind="ExternalInput").ap()

    with tile.TileContext(nc) as tc:
        with (
            tc.tile_pool(name="xpool", bufs=1) as xpool,
            tc.tile_pool(name="planes", bufs=2) as plpool,
            tc.tile_pool(name="opool", bufs=2) as opool,
        ):
            xall = xpool.tile([P, 3, COLS], dt, name="xall")
            if need_mega:
                wtile = xpool.tile([P, 64], dt, name="wtile")
                nc.sync.dma_start(out=wtile, in_=wvec.to_broadcast((P, 64)))
            t0 = schedule[0][0]
            nc.sync.dma_start(out=xall[:, :, 0:t0], in_=xin[:, :, 0:t0])
            nc.sync.dma_start(out=xall[:, :, t0:], in_=xin[:, :, t0:])

            ts = 0
            for it, (T, n_act, n_gp, use_mega) in enumerate(schedule):
                s = slice(ts, ts + T)
                px, py, pz = xall[:, 0, s], xall[:, 1, s], xall[:, 2, s]
                PL = plpool.tile([P, R_PL, T], dt, name="PL")
                ov = opool.tile([P, 64, T], dt, name="ov")

                x2, y2, z2 = PL[:, R_X2, :], PL[:, R_Y2, :], PL[:, R_Z2, :]
                n2 = PL[:, R_N2, :]

                def bc(ap, rows):
                    return ap.unsqueeze(1).broadcast_to((P, rows, T))

                # --- DVE planes (merged multi-row ops where rows align)
                # squares x2,y2,z2 and products xy,yz in one op each
                nc.vector.tensor_mul(
                    PL[:, 0:3, :], xall[:, 0:3, s], xall[:, 0:3, s]
                )
                nc.vector.tensor_mul(
                    PL[:, BASE0 : BASE0 + 2, :], xall[:, 0:2, s],
                    xall[:, 1:3, s]
                )
                nc.vector.tensor_mul(PL[:, BASE0 + 3, :], px, pz)  # zx (m7)
                nc.vector.tensor_add(PL[:, R_N2A, :], x2, y2)
                nc.vector.tensor_add(n2, PL[:, R_N2A, :], z2)
                nc.vector.tensor_sub(PL[:, BASE0 + 4, :], x2, y2)  # x2my2
                # t2a = 3*z2 - n2 (m6); a3 = 3x2-y2; b3 = x2-3y2;
                # c3 = 5z2-n2; d3 = z2-0.6n2
                nc.vector.scalar_tensor_tensor(
                    PL[:, BASE0 + 2, :], z2, 3.0, n2,
                    op0=ALU.mult, op1=ALU.subtract,
                )
                nc.vector.scalar_tensor_tensor(
                    PL[:, R_A3, :], x2, 3.0, y2, op0=ALU.mult, op1=ALU.subtract
                )
                nc.vector.scalar_tensor_tensor(
                    PL[:, R_B3, :], y2, -3.0, x2, op0=ALU.mult, op1=ALU.add
                )
                nc.vector.scalar_tensor_tensor(
                    PL[:, R_C3, :], z2, 5.0, n2, op0=ALU.mult, op1=ALU.subtract
                )
                nc.vector.scalar_tensor_tensor(
                    PL[:, R_D3, :], n2, -0.6, z2, op0=ALU.mult, op1=ALU.add
                )
                # (xyz@15, zxmy@19) = pz * (xy@9, x2my2@13)   [stride 4]
                nc.vector.tensor_mul(
                    PL[:, BASE0 + 6 : BASE0 + 11 : 4, :], bc(pz, 2),
                    PL[:, BASE0 : BASE0 + 5 : 4, :],
                )
                # (ya@14, yc@16) = py * (a3@5, c3@6)          [out stride 2]
                nc.vector.tensor_mul(
                    PL[:, BASE0 + 5 : BASE0 + 8 : 2, :], bc(py, 2),
                    PL[:, R_A3 : R_C3 + 1, :],
                )
                # (xc@18, xb@20) = px * (c3@6, b3@7)          [out stride 2]
                nc.vector.tensor_mul(
                    PL[:, BASE0 + 9 : BASE0 + 12 : 2, :], bc(px, 2),
                    PL[:, R_C3 : R_B3 + 1, :],
                )
                nc.vector.tensor_mul(PL[:, BASE0 + 8, :], pz, PL[:, R_D3, :])

                # s'_u = AA1[u] + BB1[u]*n2
                for u in range(4):
                    nc.vector.tensor_scalar(
                        PL[:, R_SP + u, :], n2, float(BB1[u]), float(AA1[u]),
                        op0=ALU.mult, op1=ALU.add,
                    )
                # l=0 (ch 0..3): A0[u] + B0[u]*n2
                for u in range(4):
                    nc.vector.tensor_scalar(
                        ov[:, u, :], n2, float(B0[u]), float(A0[u]),
                        op0=ALU.mult, op1=ALU.add,
                    )
                # l=1 (ch 4..15): p_m * s'_u (order y,z,x)
                for mi, pm in enumerate((py, pz, px)):
                    nc.vector.tensor_mul(
                        ov[:, 4 + 4 * mi : 8 + 4 * mi, :], bc(pm, 4),
                        PL[:, R_SP : R_SP + 4, :],
                    )

                # --- simple channel groups m=4..15: out = base_m * w[m,u]
                act_ms = ACT_PRIORITY[:n_act]
                gp_ms = ACT_PRIORITY[len(ACT_PRIORITY) - n_gp:] if n_gp else []
                dve_ms = sorted(set(range(4, 16)) - set(act_ms) - set(gp_ms))
                for m in act_ms:
                    base = PL[:, BASE0 + m - 4, :]
                    for u in range(4):
                        nc.scalar.activation(
                            ov[:, 4 * m + u, :], base, F.Copy,
                            scale=float(wrow[m][u]),
                        )
                for m in gp_ms:
                    base = PL[:, BASE0 + m - 4, :]
                    for u in range(4):
                        nc.gpsimd.tensor_scalar(
                            ov[:, 4 * m + u, :], base, float(wrow[m][u]),
                            None, op0=ALU.mult,
                        )
                if use_mega:
                    # contiguous m-runs -> one broadcast TT op per run
                    runs = []
                    for m in dve_ms:
                        if runs and runs[-1][1] == m - 1:
                            runs[-1][1] = m
                        else:
                            runs.append([m, m])
                    for lo, hi in runs:
                        nm = hi - lo + 1
                        nc.vector.tensor_mul(
                            ov[:, 4 * lo : 4 * hi + 4, :].rearrange(
                                "p (m u) t -> p m u t", u=4
                            ),
                            PL[:, BASE0 + lo - 4 : BASE0 + hi - 3, :]
                            .unsqueeze(2).broadcast_to((P, nm, 4, T)),
                            wtile[:, 4 * lo : 4 * hi + 4]
                            .rearrange("p (m u) -> p m u", u=4)
                            .unsqueeze(3).broadcast_to((P, nm, 4, T)),
                        )
                else:
                    for m in dve_ms:
                        base = PL[:, BASE0 + m - 4, :]
                        for u in range(4):
                            nc.vector.tensor_scalar(
                                ov[:, 4 * m + u, :], base, float(wrow[m][u]),
                                None, op0=ALU.mult,
                            )

                nc.sync.dma_start(
                    out=yout[:, 64 * ts : 64 * (ts + T)], in_=ov
                )
                ts += T

    nc.compile()
    return nc


def _get_program(consts, schedule):
    key = tuple(
        consts[k].tobytes() for k in ("A0", "B0", "AA1", "BB1", "w2", "w3")
    ) + (tuple(map(tuple, schedule)),)
    if _cache.get(key) is None:
        _cache[key] = _build_program(consts, schedule)
    return _cache[key]


def _run(x, W0, b0, W1, W2, W3, trace=False, schedule=None):
    schedule = [tuple(s) for s in (schedule or SCHEDULE)]
    consts = _host_constants(
        np.asarray(W0, np.float32), np.asarray(b0, np.float32),
        np.asarray(W1, np.float32), np.asarray(W2, np.float32),
        np.asarray(W3, np.float32),
    )
    nc = _get_program(consts, schedule)
    x = np.asarray(x, dtype=np.float32)
    shards = np.ascontiguousarray(
        x.reshape(M_CORES, P, COLS, 3).transpose(0, 1, 3, 2)
    ).astype(np.float16)
    in_maps = [{"xin": shards[c]} for c in range(M_CORES)]
    if any(s[3] for s in schedule):
        wv = consts["w64"].astype(np.float16).reshape(1, 64)
        for m in in_maps:
            m["wvec"] = wv
    kwargs = {}
    if trace:
        kwargs = dict(trace=True, trace_cores=[0])
    res = run_bass_kernel_spmd(nc, in_maps, list(range(M_CORES)), **kwargs)
    out = np.empty((M_CORES, P, COLS, 64), dtype=np.float32)
    for c in range(M_CORES):
        arr = np.asarray(res.results[c]["yout"]).reshape(P, COLS * 64)
        ts = 0
        for T, _, _, _ in schedule:
            out[c, :, ts : ts + T, :] = (
                arr[:, 64 * ts : 64 * (ts + T)]
                .reshape(P, 64, T)
                .transpose(0, 2, 1)
                .astype(np.float32)
            )
            ts += T
    return out.reshape(B, N, 16, 4), res


def kernel(x, W0, b0, W1, W2, W3):
    out, _ = _run(x, W0, b0, W1, W2, W3)
    return out


def kernel_traced(x, W0, b0, W1, W2, W3, schedule=None):
    """Like kernel(), but captures an NTFF profile; returns (out, results)."""
    import sys
    import types

    if "antenv.axon_hooks" not in sys.modules:
        mod = types.ModuleType("antenv.axon_hooks")
        _h = [None]
        mod.set_axon_ntff_profile_hook = lambda h: _h.__setitem__(0, h)
        mod.get_axon_ntff_profile_hook = lambda: _h[0]
        sys.modules["antenv.axon_hooks"] = mod
        if "/root/.axon_site" not in sys.path:
            sys.path.insert(0, "/root/.axon_site")
        from trn_agent_boot.trn_boot import _ntff_profile_via_ctypes

        mod.set_axon_ntff_profile_hook(
            _ntff_profile_via_ctypes("/opt/axon/libaxon_pjrt.so")
        )
    import concourse.bass_utils as bu

    bu.upload_artifacts = lambda tmpdir: "local://" + tmpdir
    return _run(x, W0, b0, W1, W2, W3, trace=True, schedule=schedule)
